# revision 32
# baseline (speedup 1.0000x reference)
"""Trainium2 Bass kernel for nn_CVKANTimeSeries.

Reference computation (per batch element b, sequence s, channel d):
  - complex embedding zr/zi = x @ er_w/ei_w + bias, rotated by positional
    phases (cos/sin tables).
  - 4 stacked "polarizing" layers: causal cumulative mean -> magnitude/phase
    -> tiny 1->32->1 (psi_mag) and 2->32->2 (psi_phase) GELU MLPs ->
    residual add of the polarized vector.
  - decode: gelu(zr @ op_w1 + op_b1) @ op_w2 + op_b2.

Sharding: data-parallel over batch (B=8 -> 1 batch element per NeuronCore).

Per-core layout: channels d (256) on partitions as two d-tiles of 128,
sequence s (1024) along the free dimension.  The causal cumsum is a native
DVE tensor_tensor_scan along the free dim (fp32).  The tiny MLPs run on the
tensor engine in bf16 with block-diagonal "selector" weight patterns: 4
elements per PE column, full 128-row output (L1 expands 4 channels x 32
hidden; L2 reduces back with an identity-aligned M=32 output so psum
accumulates a dense [128, n] delta).  The embedding and decode matmuls use
3-term bf16-split accumulation (hi/lo) for near-fp32 precision.
sqrt/recip/rsqrt are computed via exp/log (the ACT Reciprocal/Rsqrt tables
are banned for accuracy, and exp+ln share one ACT table set).

The module builder supports emitting the whole compute body `reps` times
(state is recomputed from DRAM each rep), used to measure on-device time by
wall-clock differencing through the high-overhead axon RPC path.
"""

import os

import ml_dtypes
import numpy as np

import concourse.bacc as bacc
import concourse.bass as bass
import concourse.mybir as mybir
import concourse.tile as tile
from concourse.bass_utils import run_bass_kernel_spmd

F32 = mybir.dt.float32
BF16 = mybir.dt.bfloat16
AF = mybir.ActivationFunctionType
ALU = mybir.AluOpType
NPBF = ml_dtypes.bfloat16

B, S, D, H, IN, L = 8, 1024, 256, 32, 64, 4
NCORES = 8
T = 2          # d-tiles of 128 partitions
NBLK = 2       # 512-column blocks of the free (s) dim
EPS_MAG = 1e-6

_BUILT = {}         # reps -> Bass module
LAST_RESULT = None  # BassKernelResults of the most recent run (for profiling)


def _build_module(reps=1):
    """Emit the Bass/Tile IR (shapes only; weights arrive via DRAM)."""
    nc = bacc.Bacc("TRN2", debug=False, num_devices=NCORES)

    dram = {}

    def din(name, shape, dt=F32):
        dram[name] = nc.dram_tensor(name, shape, dt, kind="ExternalInput")
        return dram[name]

    din("xaug_h", [IN + 1, S], BF16)
    din("xaug_l", [IN + 1, S], BF16)
    din("c_embw_rh", [IN + 1, D], BF16)
    din("c_embw_rl", [IN + 1, D], BF16)
    din("c_embw_ih", [IN + 1, D], BF16)
    din("c_embw_il", [IN + 1, D], BF16)
    din("c_rot_c", [128, T * S])
    din("c_rot_s", [128, T * S])
    din("c_invcnt", [128, S])
    din("c_w1m", [128, L * 1024], BF16)
    din("c_w1pa", [128, L * 1024], BF16)
    din("c_w1pc", [128, L * 1024], BF16)
    din("c_w2m", [128, L * 256], BF16)
    din("c_w2u", [128, L * 256], BF16)
    din("c_w2v", [128, L * 256], BF16)
    din("c_gbias", [128, 2 * L])
    din("c_scal", [1, 16])  # b2l per layer, bu/bv per layer, op_b2, eps
    din("c_dec1h", [128, T * H], BF16)
    din("c_dec1l", [128, T * H], BF16)
    din("c_dec2h", [H, 1], BF16)
    din("c_dec2l", [H, 1], BF16)
    din("c_decb1", [H, 1])
    out_dram = nc.dram_tensor("out", [1, S], F32, kind="ExternalOutput")

    with tile.TileContext(nc) as tc:
        with tc.tile_pool(name="persist", bufs=1) as persist:
            # ---- persistent constants ----
            invcnt = persist.tile([128, S], F32)
            nc.sync.dma_start(out=invcnt, in_=dram["c_invcnt"].ap())
            w2m = persist.tile([128, L * 256], BF16)
            nc.sync.dma_start(out=w2m, in_=dram["c_w2m"].ap())
            w2u = persist.tile([128, L * 256], BF16)
            nc.sync.dma_start(out=w2u, in_=dram["c_w2u"].ap())
            w2v = persist.tile([128, L * 256], BF16)
            nc.sync.dma_start(out=w2v, in_=dram["c_w2v"].ap())
            gbias = persist.tile([128, 2 * L], F32)
            nc.sync.dma_start(out=gbias, in_=dram["c_gbias"].ap())
            dec1h = persist.tile([128, T * H], BF16)
            nc.sync.dma_start(out=dec1h, in_=dram["c_dec1h"].ap())
            dec1l = persist.tile([128, T * H], BF16)
            nc.sync.dma_start(out=dec1l, in_=dram["c_dec1l"].ap())
            dec2h = persist.tile([H, 1], BF16)
            nc.sync.dma_start(out=dec2h, in_=dram["c_dec2h"].ap())
            dec2l = persist.tile([H, 1], BF16)
            nc.sync.dma_start(out=dec2l, in_=dram["c_dec2l"].ap())
            decb1 = persist.tile([H, 1], F32)
            nc.sync.dma_start(out=decb1, in_=dram["c_decb1"].ap())
            xh = persist.tile([IN + 1, S], BF16)
            nc.sync.dma_start(out=xh, in_=dram["xaug_h"].ap())
            xl = persist.tile([IN + 1, S], BF16)
            nc.sync.dma_start(out=xl, in_=dram["xaug_l"].ap())
            ewrh = persist.tile([IN + 1, D], BF16)
            nc.sync.dma_start(out=ewrh, in_=dram["c_embw_rh"].ap())
            ewrl = persist.tile([IN + 1, D], BF16)
            nc.sync.dma_start(out=ewrl, in_=dram["c_embw_rl"].ap())
            ewih = persist.tile([IN + 1, D], BF16)
            nc.sync.dma_start(out=ewih, in_=dram["c_embw_ih"].ap())
            ewil = persist.tile([IN + 1, D], BF16)
            nc.sync.dma_start(out=ewil, in_=dram["c_embw_il"].ap())
            rot_c = persist.tile([128, T * S], F32)
            nc.sync.dma_start(out=rot_c, in_=dram["c_rot_c"].ap())
            rot_s = persist.tile([128, T * S], F32)
            nc.sync.dma_start(out=rot_s, in_=dram["c_rot_s"].ap())

            # broadcast row of c_scal to 128 partitions for bias APs
            scal_b = persist.tile([128, 16], F32)
            nc.sync.dma_start(
                out=scal_b,
                in_=bass.AP(
                    tensor=dram["c_scal"].ap().tensor,
                    offset=dram["c_scal"].ap().offset,
                    ap=[[0, 128], [1, 16]],
                ),
            )

            # ---- state ----
            zr = [persist.tile([128, S], F32, name=f"zr{t}") for t in range(T)]
            zi = [persist.tile([128, S], F32, name=f"zi{t}") for t in range(T)]

            with tc.tile_pool(name="w1pool", bufs=2) as w1pool, \
                 tc.tile_pool(name="work", bufs=1) as work, \
                 tc.tile_pool(name="hsb", bufs=2) as hsb, \
                 tc.tile_pool(name="allt", bufs=1) as allt, \
                 tc.tile_pool(name="psh", bufs=1, space="PSUM") as psh, \
                 tc.tile_pool(name="psacc", bufs=1, space="PSUM") as psacc:

                for _rep in range(reps):
                    _emit_body(
                        nc, tc, dram, out_dram,
                        invcnt, w2m, w2u, w2v, gbias, scal_b,
                        dec1h, dec1l, dec2h, dec2l, decb1,
                        xh, xl, ewrh, ewrl, ewih, ewil, rot_c, rot_s,
                        zr, zi, w1pool, work, hsb, allt, psh, psacc,
                    )

    nc.compile()
    return nc


def _emit_body(nc, tc, dram, out_dram,
               invcnt, w2m, w2u, w2v, gbias, scal_b,
               dec1h, dec1l, dec2h, dec2l, decb1,
               xh, xl, ewrh, ewrl, ewih, ewil, rot_c, rot_s,
               zr, zi, w1pool, work, hsb, allt, psh, psacc):
    # ---- embedding + rotation (3-term bf16-split matmuls) ----
    for t in range(T):
        dcol = slice(128 * t, 128 * t + 128)
        for n in range(NBLK):
            cs = slice(512 * n, 512 * n + 512)
            tcs = slice(S * t + 512 * n, S * t + 512 * n + 512)
            ps_er = psh.tile([128, 512], F32, tag="hm", bufs=2, name="ps_er")
            ps_ei = psh.tile([128, 512], F32, tag="hp", bufs=3, name="ps_ei")
            for ps, wh, wl in ((ps_er, ewrh, ewrl), (ps_ei, ewih, ewil)):
                nc.tensor.matmul(ps, wh[:, dcol], xh[:, cs],
                                 start=True, stop=False)
                nc.tensor.matmul(ps, wh[:, dcol], xl[:, cs],
                                 start=False, stop=False)
                nc.tensor.matmul(ps, wl[:, dcol], xh[:, cs],
                                 start=False, stop=True)
            t1 = work.tile([128, 512], F32, tag="embt1", bufs=2, name="t1")
            t2 = work.tile([128, 512], F32, tag="embt2", bufs=2, name="t2")
            nc.vector.tensor_tensor(out=t1, in0=ps_er, in1=rot_c[:, tcs], op=ALU.mult)
            nc.vector.tensor_tensor(out=t2, in0=ps_ei, in1=rot_s[:, tcs], op=ALU.mult)
            nc.vector.tensor_tensor(out=zr[t][:, cs], in0=t1, in1=t2, op=ALU.subtract)
            nc.vector.tensor_tensor(out=t1, in0=ps_er, in1=rot_s[:, tcs], op=ALU.mult)
            nc.vector.tensor_tensor(out=t2, in0=ps_ei, in1=rot_c[:, tcs], op=ALU.mult)
            nc.vector.tensor_tensor(out=zi[t][:, cs], in0=t1, in1=t2, op=ALU.add)

    # ---- layers ----
    for l in range(L):
        w1m = w1pool.tile([128, 1024], BF16, tag="w1m", name="w1m")
        nc.sync.dma_start(out=w1m, in_=dram["c_w1m"].ap()[:, 1024 * l:1024 * l + 1024])
        w1pa = w1pool.tile([128, 1024], BF16, tag="w1pa", name="w1pa")
        nc.sync.dma_start(out=w1pa, in_=dram["c_w1pa"].ap()[:, 1024 * l:1024 * l + 1024])
        w1pc = w1pool.tile([128, 1024], BF16, tag="w1pc", name="w1pc")
        nc.sync.dma_start(out=w1pc, in_=dram["c_w1pc"].ap()[:, 1024 * l:1024 * l + 1024])

        lmf = []   # fp32 log-magnitude (for lmo)
        lmb = []   # bf16 copies for matmul rhs
        ppb = []
        qqb = []
        # ---- phase A: causal mean, magnitude, unit phase ----
        for t in range(T):
            Ar = work.tile([128, S], F32, tag="Ar", bufs=2, name="Ar")
            Ai = work.tile([128, S], F32, tag="Ai", bufs=2, name="Ai")
            sq = work.tile([128, S], F32, tag="sq", bufs=2, name="sq")
            tb = work.tile([128, S], F32, tag="tb", bufs=2, name="tb")
            lmt = work.tile([128, S], F32, tag=f"lm{t}", name="lmt")
            nc.vector.tensor_tensor_scan(
                out=Ar, data0=zr[t], data1=zr[t],
                initial=0.0, op0=ALU.add, op1=ALU.bypass,
            )
            nc.vector.tensor_tensor(out=Ar, in0=Ar, in1=invcnt, op=ALU.mult)
            nc.vector.tensor_tensor_scan(
                out=Ai, data0=zi[t], data1=zi[t],
                initial=0.0, op0=ALU.add, op1=ALU.bypass,
            )
            nc.vector.tensor_tensor(out=Ai, in0=Ai, in1=invcnt, op=ALU.mult)
            nc.vector.tensor_tensor(out=sq, in0=Ar, in1=Ar, op=ALU.mult)
            nc.vector.tensor_tensor(out=tb, in0=Ai, in1=Ai, op=ALU.mult)
            nc.vector.tensor_tensor(out=sq, in0=sq, in1=tb, op=ALU.add)
            # mag = exp(0.5*ln(m2)); lm = ln(mag+eps); inv = exp(-lm)
            nc.scalar.activation(tb, sq, AF.Ln)
            nc.scalar.activation(sq, tb, AF.Exp, scale=0.5)
            nc.scalar.activation(lmt, sq, AF.Ln, bias=scal_b[:, 13:14])
            nc.scalar.activation(tb, lmt, AF.Exp, scale=-1.0)
            lmtb = work.tile([128, S], BF16, tag=f"lmb{t}", name="lmtb")
            nc.vector.tensor_copy(out=lmtb, in_=lmt)
            pt = work.tile([128, S], BF16, tag=f"pb{t}", name="pt")
            nc.vector.tensor_tensor(out=pt, in0=Ar, in1=tb, op=ALU.mult)
            qt = work.tile([128, S], BF16, tag=f"qb{t}", name="qt")
            nc.vector.tensor_tensor(out=qt, in0=Ai, in1=tb, op=ALU.mult)
            lmf.append(lmt)
            lmb.append(lmtb)
            ppb.append(pt)
            qqb.append(qt)

        u_all = allt.tile([128, T * S], F32, tag="u_all", name="u_all")
        v_all = allt.tile([128, T * S], F32, tag="v_all", name="v_all")
        lmo_all = allt.tile([128, T * S], F32, tag="lmo_all", name="lmo_all")
        nn_all = allt.tile([128, T * S], F32, tag="nn_all", name="nn_all")

        # ---- phase B: the two tiny MLPs via PE (bf16) ----
        for t in range(T):
            for n in range(NBLK):
                blk = slice(512 * (2 * t + n), 512 * (2 * t + n) + 512)
                cs = slice(512 * n, 512 * n + 512)
                ps_d = psacc.tile([128, 512], F32, tag="d", name="ps_d")
                ps_u = psacc.tile([128, 512], F32, tag="u", name="ps_u")
                ps_v = psacc.tile([128, 512], F32, tag="v", name="ps_v")
                def flush_p(unit):
                    hp, rs, g = unit
                    sp = hsb.tile([128, 512], BF16, tag="sp", bufs=6, name="sp")
                    nc.scalar.activation(sp, hp, AF.Gelu, bias=gbias[:, 2 * l + 1:2 * l + 2])
                    w2c = slice(256 * l + 32 * g, 256 * l + 32 * g + 32)
                    nc.tensor.matmul(
                        ps_u[rs, :], w2u[:, w2c], sp,
                        start=(g == 0), stop=(g == 7),
                        skip_group_check=True,
                        tile_position=(0, rs.start),
                    )
                    nc.tensor.matmul(
                        ps_v[rs, :], w2v[:, w2c], sp,
                        start=(g == 0), stop=(g == 7),
                        skip_group_check=True,
                        tile_position=(0, rs.start),
                    )

                def flush_m(unit):
                    hm, rs, g = unit
                    sm = hsb.tile([128, 512], BF16, tag="sm", bufs=6, name="sm")
                    nc.scalar.activation(sm, hm, AF.Gelu, bias=gbias[:, 2 * l:2 * l + 1])
                    w2c = slice(256 * l + 32 * g, 256 * l + 32 * g + 32)
                    nc.tensor.matmul(
                        ps_d[rs, :], w2m[:, w2c], sm,
                        start=(g == 0), stop=(g == 7),
                        skip_group_check=True,
                        tile_position=(0, rs.start),
                    )

                # g-outer / r-inner: consecutive L1 matmuls land on different
                # 32-row strips (tile_position row groups 0/32/64/96), enabling
                # the PE's per-subarray concurrency and LDWEIGHTS pull-ahead.
                # Per psum region (strip), g still ascends 0..7, so the
                # start/stop accumulation flags remain correct.
                pend_m = []
                pend_p = []
                for g in range(8):
                    wcol = slice(128 * g, 128 * g + 128)
                    for r in range(4):
                        rs = slice(32 * r, 32 * r + 32)
                        hm = psh.tile([128, 512], F32, tag="hm", bufs=2, name="hm")
                        hp = psh.tile([128, 512], F32, tag="hp", bufs=3, name="hp")
                        nc.tensor.matmul(
                            hm, w1m[rs, wcol],
                            lmb[t][rs, cs], start=True, stop=True,
                            tile_position=(32 * r, 0),
                        )
                        nc.tensor.matmul(
                            hp, w1pa[rs, wcol],
                            ppb[t][rs, cs], start=True, stop=False,
                            tile_position=(32 * r, 0),
                        )
                        nc.tensor.matmul(
                            hp, w1pc[rs, wcol],
                            qqb[t][rs, cs], start=False, stop=True,
                            tile_position=(32 * r, 0),
                        )
                        pend_m.append((hm, rs, g))
                        pend_p.append((hp, rs, g))
                        if len(pend_p) >= 3:
                            flush_p(pend_p.pop(0))
                        if len(pend_m) >= 2:
                            flush_m(pend_m.pop(0))
                for unit in pend_p:
                    flush_p(unit)
                for unit in pend_m:
                    flush_m(unit)
                # drain psums to SBUF (+tiny-MLP output biases)
                nc.vector.tensor_scalar(
                    out=u_all[:, blk], in0=ps_u,
                    scalar1=scal_b[:, 4 + l:4 + l + 1], scalar2=None, op0=ALU.add,
                )
                nc.vector.tensor_scalar(
                    out=v_all[:, blk], in0=ps_v,
                    scalar1=scal_b[:, 8 + l:8 + l + 1], scalar2=None, op0=ALU.add,
                )
                nc.vector.scalar_tensor_tensor(
                    out=lmo_all[:, blk], in0=ps_d, scalar=1.0,
                    in1=lmf[t][:, cs], op0=ALU.mult, op1=ALU.add,
                )
                nsq = work.tile([128, 512], F32, tag="nsq", bufs=2, name="nsq")
                nc.vector.tensor_tensor(out=nn_all[:, blk], in0=u_all[:, blk], in1=u_all[:, blk], op=ALU.mult)
                nc.vector.tensor_tensor(out=nsq, in0=v_all[:, blk], in1=v_all[:, blk], op=ALU.mult)
                nc.vector.tensor_tensor(out=nn_all[:, blk], in0=nn_all[:, blk], in1=nsq, op=ALU.add)

        # ---- layer tail: r/nrm and residual update ----
        # ln(n2) in place of nn_all; rin in place of lmo_all
        nc.scalar.activation(nn_all, nn_all, AF.Ln)
        nc.vector.scalar_tensor_tensor(
            out=lmo_all, in0=nn_all, scalar=-0.5,
            in1=lmo_all, op0=ALU.mult, op1=ALU.add,
        )
        rin_all = lmo_all
        # rin = exp(lm + delta + b2l - 0.5*ln(n2)) = r / nrm
        nc.scalar.activation(rin_all, lmo_all, AF.Exp, bias=scal_b[:, l:l + 1])
        for t in range(T):
            tcs = slice(S * t, S * t + S)
            tmp = work.tile([128, S], F32, tag="updt", bufs=2, name="tmp")
            nc.vector.tensor_tensor(out=tmp, in0=rin_all[:, tcs], in1=u_all[:, tcs], op=ALU.mult)
            nc.vector.tensor_tensor(out=zr[t], in0=zr[t], in1=tmp, op=ALU.add)
            nc.vector.tensor_tensor(out=tmp, in0=rin_all[:, tcs], in1=v_all[:, tcs], op=ALU.mult)
            nc.vector.tensor_tensor(out=zi[t], in0=zi[t], in1=tmp, op=ALU.add)

    # ---- decode (3-term bf16 splits) ----
    zrh = [work.tile([128, S], BF16, tag=f"zrh{t}", name=f"zrh{t}") for t in range(T)]
    zrl = [work.tile([128, S], BF16, tag=f"zrl{t}", name=f"zrl{t}") for t in range(T)]
    for t in range(T):
        nc.vector.tensor_copy(out=zrh[t], in_=zr[t])
        nc.vector.tensor_tensor(out=zrl[t], in0=zr[t], in1=zrh[t], op=ALU.subtract)
    hd = work.tile([H, S], F32, tag="hd", name="hd")
    for n in range(NBLK):
        cs = slice(512 * n, 512 * n + 512)
        ps_dec = psh.tile([H, 512], F32, tag="hm", bufs=2, name="ps_dec")
        for t in range(T):
            hcol = slice(H * t, H * t + H)
            nc.tensor.matmul(ps_dec, dec1h[:, hcol], zrh[t][:, cs],
                             start=(t == 0), stop=False)
            nc.tensor.matmul(ps_dec, dec1h[:, hcol], zrl[t][:, cs],
                             start=False, stop=False)
            nc.tensor.matmul(ps_dec, dec1l[:, hcol], zrh[t][:, cs],
                             start=False, stop=(t == T - 1))
        nc.scalar.activation(hd[:, cs], ps_dec, AF.Gelu, bias=decb1)
    hdh = work.tile([H, S], BF16, tag="hdh", name="hdh")
    hdl = work.tile([H, S], BF16, tag="hdl", name="hdl")
    nc.vector.tensor_copy(out=hdh, in_=hd)
    nc.vector.tensor_tensor(out=hdl, in0=hd, in1=hdh, op=ALU.subtract)
    preds = work.tile([1, S], F32, tag="preds", name="preds")
    for n in range(NBLK):
        cs = slice(512 * n, 512 * n + 512)
        ps_out = psh.tile([1, 512], F32, tag="hp", bufs=3, name="ps_out")
        nc.tensor.matmul(ps_out, dec2h, hdh[:, cs], start=True, stop=False)
        nc.tensor.matmul(ps_out, dec2h, hdl[:, cs], start=False, stop=False)
        nc.tensor.matmul(ps_out, dec2l, hdh[:, cs], start=False, stop=True)
        nc.scalar.activation(preds[:, cs], ps_out, AF.Identity, bias=scal_b[0:1, 12:13])
    nc.sync.dma_start(out=out_dram.ap(), in_=preds)


def _split_bf16(a):
    hi = a.astype(NPBF)
    lo = (a - hi.astype(np.float32)).astype(NPBF)
    return hi, lo


def _prep_consts(inputs):
    """Build all weight-derived constant arrays (host side, numpy)."""
    f32 = np.float32
    er_w = np.asarray(inputs["er_w"], f32)
    er_b = np.asarray(inputs["er_b"], f32)
    ei_w = np.asarray(inputs["ei_w"], f32)
    ei_b = np.asarray(inputs["ei_b"], f32)
    pm_w1 = np.asarray(inputs["pm_w1"], f32)
    pm_b1 = np.asarray(inputs["pm_b1"], f32)
    pm_w2 = np.asarray(inputs["pm_w2"], f32)
    pm_b2 = np.asarray(inputs["pm_b2"], f32)
    pp_w1 = np.asarray(inputs["pp_w1"], f32)
    pp_b1 = np.asarray(inputs["pp_b1"], f32)
    pp_w2 = np.asarray(inputs["pp_w2"], f32)
    pp_b2 = np.asarray(inputs["pp_b2"], f32)
    mag_scale = np.asarray(inputs["mag_scale"], f32)
    op_w1 = np.asarray(inputs["op_w1"], f32)
    op_b1 = np.asarray(inputs["op_b1"], f32)
    op_w2 = np.asarray(inputs["op_w2"], f32)
    op_b2 = np.asarray(inputs["op_b2"], f32)

    c = {}
    embr = np.concatenate([er_w, er_b[None, :]], axis=0)
    embi = np.concatenate([ei_w, ei_b[None, :]], axis=0)
    c["c_embw_rh"], c["c_embw_rl"] = _split_bf16(embr)
    c["c_embw_ih"], c["c_embw_il"] = _split_bf16(embi)

    pos = np.arange(S, dtype=f32)[:, None]
    freq = np.exp(-np.log(10000.0) * np.arange(D, dtype=f32) / D).astype(f32)
    theta = (pos * freq[None, :]).astype(f32)  # [S, D]
    rc = np.cos(theta).astype(f32)
    rs = np.sin(theta).astype(f32)
    rot_c = np.empty((128, T * S), f32)
    rot_s = np.empty((128, T * S), f32)
    for t in range(T):
        rot_c[:, S * t:S * t + S] = rc[:, 128 * t:128 * t + 128].T
        rot_s[:, S * t:S * t + S] = rs[:, 128 * t:128 * t + 128].T
    c["c_rot_c"] = rot_c
    c["c_rot_s"] = rot_s

    c["c_invcnt"] = np.broadcast_to(
        (1.0 / np.arange(1, S + 1, dtype=f32))[None, :], (128, S)
    ).copy()

    # L1 selector patterns: rows k in [0,32) (strip-local channel), cols
    # g*128 + (q*32+j); value = w1[j] iff k == 4g+q.  Replicated over strips.
    def l1_pack(w1_row):
        pack = np.zeros((128, L * 1024), f32)
        for l in range(L):
            pat = np.zeros((32, 1024), f32)
            for g in range(8):
                for q in range(4):
                    pat[4 * g + q, 128 * g + 32 * q:128 * g + 32 * q + 32] = w1_row[l]
            for r in range(4):
                pack[32 * r:32 * r + 32, 1024 * l:1024 * l + 1024] = pat
        return pack.astype(NPBF)

    c["c_w1m"] = l1_pack(pm_w1[:, 0, :])
    c["c_w1pa"] = l1_pack(pp_w1[:, 0, :])
    c["c_w1pc"] = l1_pack(pp_w1[:, 1, :])

    # L2 patterns: rows (q*32+j), cols l*256 + g*32 + mo; value w2[j] iff mo==4g+q
    def l2_pack(w2_col):
        pack = np.zeros((128, L * 256), f32)
        for l in range(L):
            for g in range(8):
                for q in range(4):
                    mo = 4 * g + q
                    pack[32 * q:32 * q + 32, 256 * l + 32 * g + mo] = w2_col[l]
        return pack.astype(NPBF)

    c["c_w2m"] = l2_pack(pm_w2[:, :, 0] * mag_scale[:, None])
    c["c_w2u"] = l2_pack(pp_w2[:, :, 0])
    c["c_w2v"] = l2_pack(pp_w2[:, :, 1])

    gb = np.zeros((128, 2 * L), f32)
    for l in range(L):
        for q in range(4):
            gb[32 * q:32 * q + 32, 2 * l] = pm_b1[l]
            gb[32 * q:32 * q + 32, 2 * l + 1] = pp_b1[l]
    c["c_gbias"] = gb

    scal = np.zeros((1, 16), f32)
    scal[0, 0:4] = mag_scale * pm_b2[:, 0]      # exp bias per layer
    scal[0, 4:8] = pp_b2[:, 0]                  # u bias per layer
    scal[0, 8:12] = pp_b2[:, 1]                 # v bias per layer
    scal[0, 12] = op_b2[0]
    scal[0, 13] = EPS_MAG
    c["c_scal"] = scal

    dec1 = np.zeros((128, T * H), f32)
    for t in range(T):
        dec1[:, H * t:H * t + H] = op_w1[128 * t:128 * t + 128, :]
    c["c_dec1h"], c["c_dec1l"] = _split_bf16(dec1)
    c["c_dec2h"], c["c_dec2l"] = _split_bf16(op_w2.astype(f32))
    c["c_decb1"] = op_b1[:, None].astype(f32)
    return c


def _get_built(reps=1):
    if reps not in _BUILT:
        _BUILT[reps] = _build_module(reps)
    return _BUILT[reps]


def _make_in_maps(inputs):
    consts = _prep_consts(inputs)
    x = np.asarray(inputs["x"], np.float32)  # [B, S, IN]
    in_maps = []
    for b in range(NCORES):
        m = dict(consts)
        xaug = np.empty((IN + 1, S), np.float32)
        xaug[:IN, :] = x[b].T
        xaug[IN, :] = 1.0
        m["xaug_h"], m["xaug_l"] = _split_bf16(xaug)
        in_maps.append(m)
    return in_maps


def kernel(**inputs):
    nc = _get_built()
    in_maps = _make_in_maps(inputs)

    global LAST_RESULT
    trace = bool(int(os.environ.get("KERNEL_TRACE", "0")))
    res = run_bass_kernel_spmd(
        nc, in_maps, core_ids=list(range(NCORES)), trace=trace,
    )
    LAST_RESULT = res

    out = np.empty((B, S, 1), np.float32)
    for b in range(NCORES):
        out[b, :, 0] = res.results[b]["out"][0]
    return out
